# revision 27
# baseline (speedup 1.0000x reference)
"""Trainium2 Bass kernel for nn_CMambaSlim.

Strategy (8 NeuronCores):
  - Data-parallel trunk: each core runs the CMamba trunk (patch embed, 4
    mamba blocks, final RMSNorm) on B/8 = 4 batch samples. Trunk weights are
    bf16; rmsnorm/silu/scan pipeline is tuned for chain latency (act-table
    preload dummies, precomputed scan coefficients, bf16 DVE ops, gb folded
    into the b-projection on the host).
  - AllGather of the flattened features (bf16, 1MB) across the 8 cores.
  - Tensor-parallel output layer: core r owns output cols [r*768, (r+1)*768).
    The weight slice streams HBM->SBUF as fp8-e3m4 (x256, the 1/256 folded
    into normf_w) in [d, k, o] chunks and is the STATIONARY matmul operand
    (Ldweights is free); the gathered bf16 features are the moving operand
    (32 cols per matmul). All 25 chunks are issued on the SP queue right
    after the constant loads, so the whole 12.3MB stream prefetches into a
    25-buffer SBUF ring during the trunk + collective phase.
  The host concatenates the 8 per-core [768, 32] outputs.
"""

import math
import os
import sys

import numpy as np

for _p in ("/opt/trn_rl_repo", "/root/.axon_site/_ro/trn_rl_repo"):
    if os.path.isdir(_p) and _p not in sys.path:
        sys.path.insert(0, _p)
        break

import concourse.bass as bass
import concourse.tile as tile
from concourse import mybir
from concourse.bass_utils import run_bass_kernel_spmd

# Model dims (hardcoded per problem spec)
B, C, L = 32, 64, 512
P, S = 16, 4
NP = 125
D = 128
INNER = 256
K5 = 5
NL = 4
F = 96
EPS = 1e-5

NCORES = 8
BLOC = B // NCORES            # 4 samples per core
OSL = (C * F) // NCORES       # 768 output cols per core
TOK = BLOC * 128              # padded token span (125 valid + 3 pad per sample)
TOKW = TOK + 4                # + 2 guard cols each side
LPAD = 520                    # x padded along L so the +8-shifted copy stays in bounds
NF = NP * D                   # 16000 contraction size
KC = 5                        # k-values per streamed weight chunk
NQ = NP // KC                 # 25 chunks
NB = 25                       # weight ring depth (all chunks resident)

f32 = mybir.dt.float32
f32r = mybir.dt.float32r
bf16 = mybir.dt.bfloat16
fp8 = mybir.dt.float8e3
WT_SCALE = 256.0              # weights *256 into e3m4 range; 1/256 folded
                              # into normf_w so features shrink instead
AF = mybir.ActivationFunctionType
OP = mybir.AluOpType

_PROG = None

# --- wb (bf16) column offsets ---
WB_X = 0                                   # [128, BLOC*512] x data
WB_PE8 = WB_X + BLOC * 512                 # [128, 8*128] patch embed
WB_W5 = WB_PE8 + 8 * 128                   # [128, NL*K5*INNER] fused inproj*conv
WB_BW = WB_W5 + NL * K5 * INNER            # [128, NL*INNER] b-proj
WB_OW = WB_BW + NL * INNER                 # [128, NL*2*D] out-proj
WB_ONES = WB_OW + NL * 2 * D               # [128, 128] ones (bf16, for PE)
WBCOLS = WB_ONES + 128                     # 10368

# --- wf (f32) column offsets ---
WF_SCAL = 0                                # [128, NL*4*2] per-layer scalars
WF_MISC = WF_SCAL + NL * 4 * 2             # [128, 129] pos emb + normf_w
WF_OUTB = WF_MISC + 129                    # [128, 6] out_b blocks
WFCOLS = WF_OUTB + 6                       # 167


def build_program():
    nc = bass.Bass(num_devices=NCORES)

    wb = nc.declare_dram_parameter("wb", [128, WBCOLS], bf16, isOutput=False)
    wf = nc.declare_dram_parameter("wf", [128, WFCOLS], f32, isOutput=False)
    wt = nc.declare_dram_parameter("wt", [128, NP * OSL], fp8, isOutput=False)
    y = nc.declare_dram_parameter("y", [768, 32], f32, isOutput=True)

    wtT = wt[:].tensor

    with tile.TileContext(nc) as tc:
        with (
            tc.tile_pool(name="const", bufs=1) as const,
            tc.tile_pool(name="work", bufs=1) as work,
            tc.tile_pool(name="wring", bufs=NB) as wring,
            tc.tile_pool(name="ps", bufs=1, space="PSUM") as ps,
            tc.tile_pool(name="dram", bufs=1, space="DRAM") as dram,
        ):
            # ---------------- constant loads (SP queue, then wt ring) -------
            # wf first (small), then wb in three pieces so patch embedding
            # can start as soon as x + pe_W land (~3us) instead of ~8us.
            wfs = const.tile([128, WFCOLS], f32)
            nc.sync.dma_start(out=wfs[:], in_=wf[:])
            wbs = const.tile([128, WBCOLS], bf16)
            nc.sync.dma_start(out=wbs[:, 0:WB_W5], in_=wb[:, 0:WB_W5])
            nc.sync.dma_start(out=wbs[:, WB_W5:WB_BW], in_=wb[:, WB_W5:WB_BW])
            nc.sync.dma_start(out=wbs[:, WB_BW:WBCOLS], in_=wb[:, WB_BW:WBCOLS])

            # first NB weight chunks: no deps, so the SP queue streams them
            # from t~0 into the ring while the trunk runs. The remaining
            # chunks are emitted after the gather (their ring slots only
            # free once the matmuls drain them; issuing them before the
            # critical section's engine barrier would deadlock SP).
            wtls = []
            for q in range(NQ):
                if q >= NB:
                    break
                wtl = wring.tile([128, KC, OSL], fp8, tag="wt", name="t_wt")
                nc.sync.dma_start(
                    out=wtl[:],
                    in_=bass.AP(tensor=wtT, offset=q * KC * OSL,
                                ap=[[NP * OSL, 128], [OSL, KC], [1, OSL]]),
                )
                wtls.append(wtl)

            xO4 = wbs[:, WB_X:WB_PE8].rearrange(
                "p (b k s) -> p b k s", b=BLOC, s=4)          # [128, 4, 128, 4]
            pe8sb = wbs[:, WB_PE8:WB_W5].rearrange("p (j d) -> p j d", j=8)
            w5sb = wbs[:, WB_W5:WB_BW].rearrange(
                "p (l k i) -> p l k i", l=NL, k=K5)
            bwsb = wbs[:, WB_BW:WB_OW].rearrange("p (l i) -> p l i", l=NL)
            owsb = wbs[:, WB_OW:WB_ONES].rearrange(
                "p (l c d) -> p l c d", l=NL, c=2)
            onesD = wbs[:, WB_ONES:WB_ONES + 1]
            ones1 = wbs[0:1, WB_ONES:WB_ONES + 128]

            scalsb = wfs[:, WF_SCAL:WF_MISC].rearrange(
                "p (l s c) -> p l s c", l=NL, s=4)
            miscsb = wfs[:, WF_MISC:WF_OUTB]
            outbsb = wfs[:, WF_OUTB:WFCOLS]

            eps1 = const.tile([1, 1], f32)
            nc.vector.memset(eps1[:], EPS)

            # act-table preload: Sqrt forces the sqrt table (which also has
            # square + identity) resident before the first real activation.
            dum = const.tile([1, 1], f32)
            nc.scalar.activation(out=dum[:], in_=eps1[:], func=AF.Sqrt)

            # mask01: 1 everywhere, 0 at each sample's k=0 column (scan reset)
            mask01 = const.tile([128, TOK], bf16)
            nc.vector.memset(mask01[:], 1.0)
            for bq in range(BLOC):
                nc.vector.memset(mask01[:, bq * 128:bq * 128 + 1], 0.0)

            # precompute the masked scan coefficients for every layer up
            # front (consts only; runs while the trunk DMA streams in)
            aMl = []
            for l in range(NL):
                pair = []
                for ic in range(2):
                    t = const.tile([128, TOK], bf16, name=f"aM{l}{ic}")
                    nc.vector.tensor_scalar_mul(
                        out=t[:], in0=mask01[:],
                        scalar1=scalsb[:, l, 1, ic:ic + 1])
                    pair.append(t)
                aMl.append(pair)

            # residual stream h: [d, 2 guard + (b,k) + 2 guard], fp32
            h = const.tile([128, TOKW], f32)
            nc.vector.memset(h[:], 0.0)
            h_tok = h[:, 2:2 + TOK]
            h_bk = h_tok.rearrange("p (b k) -> p b k", b=BLOC)

            # ---------------- patch embedding ----------------
            ph = ps.tile([128, BLOC, 126], f32, tag="prstd")
            for j in range(8):
                jq, jr = j // 4, j % 4
                rhs = xO4[:, :, jq:jq + 126, jr]
                nc.tensor.matmul(
                    out=ph[:], lhsT=pe8sb[:, j, :],
                    rhs=rhs, start=(j == 0), stop=(j == 7))
            posb = miscsb[:, 0:NP].unsqueeze(1).broadcast_to([128, BLOC, NP])
            nc.vector.tensor_tensor(
                out=h_bk[:, :, 0:NP], in0=ph[:, :, 0:NP], in1=posb, op=OP.add)

            # ---------------- mamba layers ----------------
            for l in range(NL):
                # rmsnorm stats: ssum = sum_d h^2 (PE ones-reduce on squared h)
                sq = work.tile([128, TOK], bf16, tag="sq", name="t_sq")
                nc.scalar.activation(out=sq[:], in_=h_tok, func=AF.Square)
                pssum = ps.tile([1, TOK], f32, tag="py", bufs=2, name="t_pssum")
                nc.tensor.matmul(out=pssum[:], lhsT=onesD,
                                 rhs=sq[:], start=True, stop=True)
                sd = work.tile([1, TOK], f32, tag="sd", name="t_sd")
                nc.scalar.activation(out=sd[:], in_=pssum[:], func=AF.Sqrt,
                                     bias=eps1[:], scale=1.0 / D)
                rstd = work.tile([1, TOK], bf16, tag="rstd", name="t_rstd")
                with nc.allow_low_precision("bf16 rstd fine for 2e-2 tol"):
                    nc.vector.reciprocal(out=rstd[:], in_=sd[:])
                prstd = ps.tile([128, TOK], f32, tag="prstd", name="t_prstd")
                nc.tensor.matmul(out=prstd[:], lhsT=ones1,
                                 rhs=rstd[:], start=True, stop=True)
                # hn = h * rstd  (norm_w folded into w5/bw on host), bf16
                # so the PE in/b-projections run all-bf16. 2 zero guard
                # columns each side make every conv window a full 512 wide.
                hn = work.tile([128, TOKW], bf16, tag="hn", name="t_hn")
                nc.vector.tensor_tensor(out=hn[:, 0:2], in0=h[:, 0:2],
                                        in1=h[:, 0:2], op=OP.mult)
                nc.vector.tensor_tensor(out=hn[:, 2 + TOK:], in0=h[:, 2 + TOK:],
                                        in1=h[:, 2 + TOK:], op=OP.mult)
                nc.vector.tensor_tensor(out=hn[:, 2:2 + TOK], in0=h_tok,
                                        in1=prstd[:], op=OP.mult)

                # fused in-proj + depthwise conv on PE (5 shifted matmuls
                # per half); each half's sigmoid/scan starts as soon as its
                # conv finishes, overlapping the other half's matmuls.
                pa = [ps.tile([128, TOK], f32, tag=f"pa{ic}", name=f"pa{ic}_{l}") for ic in range(2)]
                ab, sc = [], []
                first_sig = True
                for ic in range(2):
                    for dk in range(K5):
                        nc.tensor.matmul(
                            out=pa[ic][:],
                            lhsT=w5sb[:, l, dk, ic * 128:(ic + 1) * 128],
                            rhs=hn[:, dk:dk + TOK],
                            start=(dk == 0), stop=(dk == K5 - 1))
                    if first_sig:
                        # dummy anchored on sd: loads the sigmoid table while
                        # the conv matmuls run (anchor stops the scheduler
                        # from hoisting it to t=0)
                        nc.scalar.activation(out=dum[:], in_=sd[0:1, 0:1],
                                             func=AF.Sigmoid)
                        first_sig = False
                    # silu(z) = z * sigmoid(z), z = conv + conv_b
                    sg = work.tile([128, TOK], f32, tag=f"sg{ic}", name=f"w{ic}_{l}")
                    nc.scalar.activation(out=sg[:], in_=pa[ic][:], func=AF.Sigmoid,
                                         bias=scalsb[:, l, 0, ic:ic + 1], scale=1.0)
                    u = work.tile([128, TOK], bf16, tag=f"ab{ic}", name=f"w{ic}_{l}")
                    nc.vector.scalar_tensor_tensor(
                        out=u[:], in0=pa[ic][:], scalar=scalsb[:, l, 0, ic:ic + 1],
                        in1=sg[:], op0=OP.add, op1=OP.mult)
                    ab.append(u)
                    # scan: state = aMask*state + u (all-bf16: 2x DVE rate)
                    s = work.tile([128, TOK], bf16, tag=f"s{ic}", name=f"w{ic}_{l}")
                    nc.vector.tensor_tensor_scan(
                        out=s[:], data0=aMl[l][ic][:], data1=u[:], initial=0.0,
                        op0=OP.mult, op1=OP.add)
                    sc.append(s)

                # b-projection (PE)
                pb = [ps.tile([128, TOK], f32, tag=f"pb{ic}", name=f"pb{ic}_{l}") for ic in range(2)]
                for ic in range(2):
                    nc.tensor.matmul(
                        out=pb[ic][:],
                        lhsT=bwsb[:, l, ic * 128:(ic + 1) * 128],
                        rhs=hn[:, 2:2 + TOK], start=True, stop=True)

                # dummy anchored on the last sigmoid output: preloads the
                # sqrt table for the next rmsnorm while the gate DVE runs
                nc.scalar.activation(out=dum[:], in_=sg[0:1, 0:1],
                                     func=AF.Sqrt)

                # gate: g = (gamma*beta*s + delta*u) * b
                g = []
                for ic in range(2):
                    dab = work.tile([128, TOK], bf16, tag=f"dab{ic}", name=f"w{ic}_{l}")
                    nc.vector.tensor_scalar_mul(
                        out=dab[:], in0=ab[ic][:], scalar1=scalsb[:, l, 3, ic:ic + 1])
                    g0 = work.tile([128, TOK], bf16, tag=f"g0{ic}", name=f"w{ic}_{l}")
                    nc.vector.tensor_tensor(out=g0[:], in0=sc[ic][:],
                                            in1=dab[:], op=OP.add)
                    gg = work.tile([128, TOK], bf16, tag=f"g{ic}", name=f"w{ic}_{l}")
                    # pads (k=125..127) are zero because hn pads are zero, so
                    # pb pads are zero and the product zeroes them.
                    nc.vector.tensor_tensor(out=gg[:], in0=g0[:], in1=pb[ic][:],
                                            op=OP.mult)
                    g.append(gg)

                # out-projection + residual: h = 2*h + oW @ g
                py = ps.tile([128, TOK], f32, tag="py", bufs=2, name="t_py")
                for ic in range(2):
                    nc.tensor.matmul(
                        out=py[:], lhsT=owsb[:, l, ic, :],
                        rhs=g[ic][:], start=(ic == 0), stop=(ic == 1))
                nc.vector.scalar_tensor_tensor(
                    out=h_tok, in0=h_tok, scalar=2.0, in1=py[:],
                    op0=OP.mult, op1=OP.add)

            # ---------------- final rmsnorm (-> bf16 features) ----------------
            sqf = work.tile([128, TOK], bf16, tag="sq", name="t_sq")
            nc.scalar.activation(out=sqf[:], in_=h_tok, func=AF.Square)
            pssumf = ps.tile([1, TOK], f32, tag="py", bufs=2, name="t_pssum")
            nc.tensor.matmul(out=pssumf[:], lhsT=onesD,
                             rhs=sqf[:], start=True, stop=True)
            sdf = work.tile([1, TOK], f32, tag="sd", name="t_sd")
            nc.scalar.activation(out=sdf[:], in_=pssumf[:], func=AF.Sqrt,
                                 bias=eps1[:], scale=1.0 / D)
            rstdf = work.tile([1, TOK], bf16, tag="rstd", name="t_rstd")
            with nc.allow_low_precision("bf16 rstd fine for 2e-2 tol"):
                nc.vector.reciprocal(out=rstdf[:], in_=sdf[:])
            prstdf = ps.tile([128, TOK], f32, tag="prstd", name="t_prstd")
            nc.tensor.matmul(out=prstdf[:], lhsT=ones1,
                             rhs=rstdf[:], start=True, stop=True)
            hf = work.tile([128, TOK], bf16, tag="hf", name="t_hf")
            nc.vector.scalar_tensor_tensor(
                out=hf[:], in0=h_tok, scalar=miscsb[:, 128:129], in1=prstdf[:],
                op0=OP.mult, op1=OP.mult)

            # ---------------- all-gather the features (bf16) ----------------
            ccin = dram.tile([128, TOK], bf16)
            nc.scalar.dma_start(out=ccin[:], in_=hf[:])
            ccout = dram.tile([NCORES, 128, TOK], bf16, addr_space="Shared")
            nc.gpsimd.collective_compute(
                "AllGather", OP.bypass,
                replica_groups=[list(range(NCORES))],
                ins=[ccin[:].opt()], outs=[ccout[:].opt()])
            # flatT[d, slot, b4*128 + k]; slot = source core rank
            flatT = const.tile([128, NCORES, TOK], bf16)
            nc.scalar.dma_start(
                out=flatT[:],
                in_=bass.AP(tensor=ccout[:].tensor, offset=ccout[:].offset,
                            ap=[[TOK, 128], [128 * TOK, NCORES], [1, TOK]]),
            )
            fr = flatT[:].rearrange("p s (b k) -> p s b k", b=BLOC)

            # ---------------- streamed output matmul (weights stationary) ----
            # yps[o_in_block + 128*j ... , (slot, b4)] accumulated over k.
            # Stationary = wt chunk [d, o] (Ldweights), moving = 32 feature
            # cols [d, (slot, b4)] at offset k.
            # pre-zero once and accumulate with start=False throughout:
            # six accumulation blocks share one PSUM zero region, and a
            # start_tensor_calc zeroes the whole 2KB region (would wipe the
            # other blocks' k=0 contribution).
            yps = ps.tile([128, 6, 32], f32, tag="yps", name="t_yps")
            nc.vector.memset(yps[:], 0.0)
            for q in range(NB, NQ):
                wtl = wring.tile([128, KC, OSL], fp8, tag="wt", name="t_wt")
                nc.sync.dma_start(
                    out=wtl[:],
                    in_=bass.AP(tensor=wtT, offset=q * KC * OSL,
                                ap=[[NP * OSL, 128], [OSL, KC], [1, OSL]]),
                )
                wtls.append(wtl)
            for q in range(NQ):
                wtl = wtls[q]
                for kc in range(KC):
                    k = q * KC + kc
                    rhs = fr[:, :, :, k]
                    for j in range(6):
                        nc.tensor.matmul(
                            out=yps[:, j, :],
                            lhsT=wtl[:, kc, j * 128:(j + 1) * 128],
                            rhs=rhs,
                            start=False, stop=(k == NP - 1),
                            skip_group_check=True)

            # bias add + copy out:  y[o, b] = yps + out_b[o]
            yout = work.tile([128, 6, 32], f32, tag="yout", name="t_yout")
            for j in range(6):
                nc.scalar.activation(out=yout[:, j, :], in_=yps[:, j, :],
                                     func=AF.Identity, bias=outbsb[:, j:j + 1],
                                     scale=1.0)
            nc.scalar.dma_start(
                out=bass.AP(tensor=y[:].tensor, offset=0,
                            ap=[[32, 128], [32 * 128, 6], [1, 32]]),
                in_=yout[:])

    _legalize_waits(nc)
    return nc


def _legalize_waits(nc):
    """walrus on this toolchain accepts only one sync wait per non-sequencer
    instruction. Move extra waits onto standalone InstEventSemaphore
    instructions (sequencer-level waits, multi-wait legal) placed just
    before the owning instruction on the same engine."""
    n_moved = 0
    for bb in nc.main_func.blocks:
        out = []
        for inst in bb.instructions:
            si = inst.sync_info
            tn = type(inst).__name__
            if (si is not None and len(si.on_wait) > 1
                    and tn not in ("InstEventSemaphore", "InstNoOp")):
                waits = list(si.on_wait)
                for w in waits[:-1]:
                    ev = mybir.InstNoOp(
                        name=f"lw_{inst.name}_{n_moved}", ins=[], outs=[],
                        engine=inst.engine)
                    ev.sync_info = mybir.SyncInfo(on_wait=[w], on_update=[])
                    nc.register_instruction(ev)
                    out.append(ev)
                    n_moved += 1
                inst.sync_info = mybir.SyncInfo(
                    on_wait=[waits[-1]], on_update=list(si.on_update))
            out.append(inst)
        bb.instructions = out


def _sincos_pe(n, d):
    pos = np.arange(n, dtype=np.float32)[:, None]
    sin_cols, cos_cols = (d + 1) // 2, d // 2
    denom = d / 2.0
    sin_div = np.exp(
        (-math.log(10000.0) * np.arange(sin_cols, dtype=np.float32) / denom)
    ).astype(np.float32)
    cos_div = np.exp(
        (-math.log(10000.0) * np.arange(cos_cols, dtype=np.float32) / denom)
    ).astype(np.float32)
    pe = np.zeros((n, d), dtype=np.float32)
    pe[:, 0::2] = np.sin(pos * sin_div[None, :])
    pe[:, 1::2] = np.cos(pos * cos_div[None, :])
    return pe


def make_in_maps(x, pe_W, pe_b, norm_w, ipa_W, ipb_W, conv_W, conv_b,
                 alpha, beta, gamma, delta, op_W, normf_w, out_W, out_b):
    import ml_dtypes
    f = np.float32
    b16 = ml_dtypes.bfloat16
    x = np.asarray(x, f)
    x_pad = np.zeros((B, C, LPAD), f)
    x_pad[:, :, :L] = x
    # device layout: [p2*64+c, b_loc, l] with p2=1 rows shifted by 8 along l
    xcT = x_pad.transpose(1, 0, 2)                     # [c, b, lpad]
    x_dev = np.empty((2, C, B, 512), f)
    x_dev[0] = xcT[:, :, 0:512]
    x_dev[1] = xcT[:, :, 8:520]
    x_dev = x_dev.reshape(128, B, 512)

    pw = np.asarray(pe_W, f).reshape(D, C, P)          # [d, c, p]
    t = pw.transpose(1, 2, 0)                          # [c, p, d]
    pe8 = np.ascontiguousarray(
        t.reshape(C, 2, 8, D).transpose(2, 1, 0, 3).reshape(8, 128, 128))
    pe8 = np.ascontiguousarray(pe8.transpose(1, 0, 2))  # [pp, j, d]

    posb = np.zeros((128, 129), f)
    posb[:, :NP] = _sincos_pe(NP, D).T + np.asarray(pe_b, f)[:, None]
    posb[:, 128] = np.asarray(normf_w, f) / WT_SCALE

    nw = np.asarray(norm_w, f)                          # [NL, D]
    ipa = np.asarray(ipa_W, f)                          # [NL, INNER, D]
    cw = np.asarray(conv_W, f)[:, :, 0, :]              # [NL, INNER, K5]
    w5 = (ipa.transpose(0, 2, 1)[:, None, :, :]         # [NL, 1, D, INNER]
          * cw.transpose(0, 2, 1)[:, :, None, :]        # [NL, K5, 1, INNER]
          * nw[:, None, :, None])
    w5 = np.ascontiguousarray(w5.transpose(2, 0, 1, 3), f)   # [d, NL, K5, INNER]
    gb64 = np.asarray(gamma, np.float64) * np.asarray(beta, np.float64)
    bwh = (np.asarray(ipb_W, f).transpose(0, 2, 1) * nw[:, :, None]
           * gb64.astype(f)[:, None, :])                     # gb folded in
    bwh = np.ascontiguousarray(bwh.transpose(1, 0, 2), f)    # [d, NL, INNER]
    owh = np.asarray(op_W, f).transpose(0, 2, 1).reshape(NL, 2, 128, D)
    owh = np.ascontiguousarray(owh.transpose(2, 0, 1, 3), f)  # [i, NL, 2, d]

    asig = 1.0 / (1.0 + np.exp(-np.asarray(alpha, np.float64)))
    gb = np.asarray(gamma, np.float64) * np.asarray(beta, np.float64)
    dl = np.asarray(delta, np.float64) / np.where(gb == 0, 1.0, gb)
    p4 = np.stack([np.asarray(conv_b, f),
                   asig.astype(f),
                   gb.astype(f),
                   dl.astype(f)], axis=0)               # [4, NL, INNER]
    scal = np.ascontiguousarray(
        p4.reshape(4, NL, 2, 128).transpose(3, 1, 0, 2), f)  # [128, NL, 4, 2]

    oW = np.asarray(out_W, f)
    ob = np.asarray(out_b, f)

    wb_shared = np.concatenate([
        pe8.reshape(128, 8 * 128), w5.reshape(128, NL * K5 * INNER),
        bwh.reshape(128, NL * INNER), owh.reshape(128, NL * 2 * D),
        np.ones((128, 128), f),
    ], axis=1).astype(b16)

    in_maps = []
    for r in range(NCORES):
        wb_r = np.concatenate([
            x_dev[:, r * BLOC:(r + 1) * BLOC, :].reshape(128, BLOC * 512
                                                         ).astype(b16),
            wb_shared,
        ], axis=1)
        wf_r = np.concatenate([
            scal.reshape(128, NL * 4 * 2), posb,
            ob[r * OSL:(r + 1) * OSL].reshape(6, 128).T.astype(f),
        ], axis=1)
        # wt[d, k, o] = out_W[r*768 + o, k*128 + d] * 256 in fp8-e3m4
        wt_r = np.clip(np.ascontiguousarray(
            oW[r * OSL:(r + 1) * OSL].reshape(OSL, NP, D
                                              ).transpose(2, 1, 0)) * WT_SCALE,
            -15.5, 15.5).astype(ml_dtypes.float8_e3m4)
        in_maps.append({
            "wb": np.ascontiguousarray(wb_r),
            "wf": np.ascontiguousarray(wf_r),
            "wt": wt_r.reshape(128, NP * OSL),
        })
    return in_maps


def kernel(**inputs):
    global _PROG
    if _PROG is None:
        _PROG = build_program()
    in_maps = make_in_maps(**inputs)
    res = run_bass_kernel_spmd(_PROG, in_maps, list(range(NCORES)))
    return assemble_output([res.results[r]["y"] for r in range(NCORES)])


def assemble_output(ys):
    # per-core y is [768, 32] = [o, (slot, b4)]; batch row = 4*slot + b4.
    y = np.concatenate(
        [np.asarray(yr, np.float32).reshape(OSL, B).T for yr in ys], axis=1)
    return y.reshape(B, C, F)
